# revision 3
# baseline (speedup 1.0000x reference)
"""Trainium2 Bass kernel v2 for nn_DotAttention: softmax(Q @ V^T) @ V.

v2 changes vs v1:
  - Host pre-transposes: qT [64, 2048] per core, vT [64, 4096], and
    v_ext [4096, 65] (ones column appended). Kills all PE setup transposes.
  - DMA'd fp32 must still be rounded to f32r by an on-chip producer
    (walrus verifier), so one DVE tensor_copy per piece does fp32->f32r.
  - VGRP=3: exp reads [128, 1536] from PSUM (10 full groups + one of 2).
  - Inputs stream in pieces so chunk-0 compute starts early.

Per-core loop (chunk = 512 q columns):
  mm1 (fp32r): scoresT[v_tile, q] in PSUM -> exp (ScalarE, PSUM->SBUF f32r)
  -> mm2 (fp32r): ctxT[65, 512] += V_ext_tile^T @ expT accumulated in PSUM
  (row 64 = softmax denominator) -> PE transpose -> normalize -> DMA out.
"""

import sys

sys.path.insert(0, "/opt/trn_rl_repo")

import numpy as np

import concourse.bass as bass  # noqa: F401
import concourse.mybir as mybir
import concourse.tile as tile
from concourse import bacc
from concourse.bass_utils import run_bass_kernel_spmd
from concourse.masks import make_identity

F32 = mybir.dt.float32
F32R = mybir.dt.float32r
EXP = mybir.ActivationFunctionType.Exp

B, TQ, TV, D = 4, 4096, 4096, 64
N_CORES = 8
QS = TQ * B // N_CORES  # 2048
CHUNK = 512
NCH = QS // CHUNK  # 4
NVT = TV // 128  # 32
VPIECE = 8  # v tiles per input piece
NVP = NVT // VPIECE  # 4 pieces

# exp groups per chunk: 10 groups of 3 v-tiles + 1 group of 2
GROUPS = [(s, min(3, NVT - s)) for s in range(0, NVT, 3)]

_cache = {}


def _build():
    nc = bacc.Bacc("TRN2", target_bir_lowering=False, debug=False)
    qt_d = nc.dram_tensor("qT", [64, QS], F32, kind="ExternalInput").ap()
    vt_d = nc.dram_tensor("vT", [64, TV], F32, kind="ExternalInput").ap()
    ve_d = nc.dram_tensor("vE", [TV, D + 1], F32, kind="ExternalInput").ap()
    out = nc.dram_tensor("out", [QS, D], F32, kind="ExternalOutput").ap()

    with tile.TileContext(nc) as tc:
        with (
            tc.tile_pool(name="const", bufs=1) as const_pool,
            tc.tile_pool(name="stage", bufs=1) as stage_pool,
            tc.tile_pool(name="big", bufs=1) as big_pool,
            tc.tile_pool(name="sb", bufs=1) as sb_pool,
            tc.tile_pool(name="ps", bufs=1, space="PSUM") as ps_pool,
            tc.tile_pool(name="pst", bufs=1, space="PSUM") as pst_pool,
        ):
            ident = const_pool.tile([128, 128], F32)
            make_identity(nc, ident)

            # PE observes the identity's (gpsimd) semaphore once so later
            # transposes carry fewer waits
            warm = pst_pool.tile([128, 128], F32, tag="tr", bufs=1)
            nc.tensor.transpose(warm, ident, ident)

            vt = big_pool.tile([64, TV], F32R, tag="vt")
            qt = big_pool.tile([64, QS], F32R, tag="qt")
            ve_list = []

            def v_piece(p):
                """DMA vT piece + v_ext piece, round to f32r."""
                c0, c1 = p * VPIECE * 128, (p + 1) * VPIECE * 128
                vs = stage_pool.tile(
                    [64, VPIECE * 128], F32, tag="vstage", bufs=2, name=f"vs{p}"
                )
                nc.sync.dma_start(out=vs, in_=vt_d[:, c0:c1])
                nc.vector.tensor_copy(vt[:, c0:c1], vs)
                es = stage_pool.tile(
                    [128, VPIECE, D + 1], F32, tag="estage", bufs=2, name=f"es{p}"
                )
                nc.sync.dma_start(
                    out=es, in_=ve_d[c0:c1, :].rearrange("(t p) e -> p t e", p=128)
                )
                ve = big_pool.tile(
                    [128, VPIECE, D + 1], F32R, tag=f"ve{p}", bufs=1, name=f"ve{p}"
                )
                ve_list.append(ve)
                nc.vector.tensor_copy(ve, es)

            def q_piece(ch):
                c0, c1 = ch * CHUNK, (ch + 1) * CHUNK
                qs_ = stage_pool.tile(
                    [64, CHUNK], F32, tag="qstage", bufs=2, name=f"qs{ch}"
                )
                nc.sync.dma_start(out=qs_, in_=qt_d[:, c0:c1])
                nc.vector.tensor_copy(qt[:, c0:c1], qs_)

            v_piece(0)
            q_piece(0)

            for ch in range(NCH):
                if ch > 0:
                    q_piece(ch)
                ctx_ps = ps_pool.tile(
                    [D + 1, CHUNK], F32, tag="ctx", bufs=1, name=f"ctx{ch}"
                )
                for gi, (g0, gn) in enumerate(GROUPS):
                    if ch == 0 and g0 in (6, 15, 21):
                        # piece p covers v tiles 8p..8p+7; prefetch one
                        # group before its first consumer
                        v_piece({6: 1, 15: 2, 21: 3}[g0])
                    sc = ps_pool.tile(
                        [128, gn, CHUNK], F32, tag="scores", bufs=2,
                        padded_shape=[128, 3, CHUNK], name=f"sc{ch}_{g0}",
                    )
                    for t in range(gn):
                        i = g0 + t
                        nc.tensor.matmul(
                            sc[:, t, :],
                            vt[:, i * 128 : (i + 1) * 128],
                            qt[:, ch * CHUNK : (ch + 1) * CHUNK],
                            start=True,
                            stop=True,
                        )
                    et = sb_pool.tile(
                        [128, gn, CHUNK], F32R, tag="expt", bufs=3,
                        padded_shape=[128, 3, CHUNK], name=f"et{ch}_{g0}",
                    )
                    nc.scalar.activation(et, sc, EXP)
                    for t in range(gn):
                        i = g0 + t
                        nc.tensor.matmul(
                            ctx_ps,
                            ve_list[i // VPIECE][:, i % VPIECE, :],
                            et[:, t, :],
                            start=(i == 0),
                            stop=(i == NVT - 1),
                        )
                # epilogue
                ctxt = sb_pool.tile(
                    [D + 1, CHUNK], F32, tag="ctxt", bufs=2, name=f"ctxt{ch}"
                )
                nc.vector.tensor_copy(ctxt, ctx_ps)
                for j in range(CHUNK // 128):
                    tr = pst_pool.tile(
                        [128, D + 1], F32, tag="tr", bufs=1, name=f"tr{ch}_{j}"
                    )
                    nc.tensor.transpose(
                        tr, ctxt[:, j * 128 : (j + 1) * 128], ident[: D + 1, : D + 1]
                    )
                    o_t = sb_pool.tile([128, D + 1], F32, tag="ot", bufs=2)
                    nc.vector.tensor_copy(o_t, tr)
                    rec = sb_pool.tile([128, 1], F32, tag="rec", bufs=2)
                    nc.vector.reciprocal(rec, o_t[:, D : D + 1])
                    o_n = sb_pool.tile([128, D], F32, tag="on", bufs=2)
                    nc.vector.tensor_scalar_mul(o_n, o_t[:, :D], rec)
                    nc.sync.dma_start(
                        out=out[ch * CHUNK + j * 128 : ch * CHUNK + (j + 1) * 128, :],
                        in_=o_n,
                    )

    nc.compile()
    return nc


def _get_nc():
    if "nc" not in _cache:
        _cache["nc"] = _build()
    return _cache["nc"]


def kernel(query: np.ndarray, value: np.ndarray, **run_kwargs) -> np.ndarray:
    query = np.asarray(query, dtype=np.float32)
    value = np.asarray(value, dtype=np.float32)
    nc = _get_nc()
    shards_per_b = N_CORES // B
    in_maps = []
    ones = np.ones((TV, 1), dtype=np.float32)
    for c in range(N_CORES):
        b, s = divmod(c, shards_per_b)
        in_maps.append(
            {
                "qT": np.ascontiguousarray(query[b, s * QS : (s + 1) * QS].T),
                "vT": np.ascontiguousarray(value[b].T),
                "vE": np.ascontiguousarray(
                    np.concatenate([value[b], ones], axis=1)
                ),
            }
        )
    res = run_bass_kernel_spmd(nc, in_maps, core_ids=list(range(N_CORES)), **run_kwargs)
    _cache["last_results"] = res
    out = np.empty((B, TQ, D), dtype=np.float32)
    for c in range(N_CORES):
        b, s = divmod(c, shards_per_b)
        out[b, s * QS : (s + 1) * QS] = res.results[c]["out"]
    return out
